# revision 19
# baseline (speedup 1.0000x reference)
"""Cross multi-head attention on 8 Trainium2 NeuronCores.

Problem: y = CrossMHA(x, memory) with B=4, Tq=1024, Tk=2048, D=1024, H=16.

Sharding: 8 cores = (batch b in 0..3) x (head-half s in 0..1).  Each core
handles one batch element and 8 of the 16 heads: it computes the q/k/v
projections for its head columns, attention for its heads, and a partial
output projection y_partial = ctx @ W_o[rows of its heads].  The host sums
the two head-half partials per batch and adds b_o.

Per-core kernel (all matmul operands float32r -> full PE rate, fp32 PSUM):
  - Qt = W_q_s^T @ x^T          [512, 1024]  (q transposed: heads on partitions)
  - Kt = W_k_s^T @ mem^T        [512, 2048]
  - V  = mem @ W_v_s            [2048, 8, 65] (65th column = ones, so that the
                                A@V matmul also produces the softmax denom)
  - per (head, q-tile of 512):
      St[k,q]   = Kt_h^T-slices @ Qt_h     (keys on partitions, 16 k-chunks,
                                            two chunks share one PSUM tile)
      Ae        = exp(St/8 + mask_bias[k]) (ACT pass over the chunk pair; the
                                            mask bias is per-partition here)
      ctx/den   = V_aug^T @ Ae             (accumulated over the 16 k-chunks;
                                            row 64 is the softmax denominator)
      ctx_norm  = ctx * (1/den)            (1/den broadcast via tiny matmul)
  - y_partial = ctx_norm^T @ W_o_rows      [1024, 1024]

Schedule: the first attention group is interleaved with the last two K/V
projection k-tiles so the scalar engine (exp) starts early; the output
projection for the first q-half runs while the second q-half's attention
groups still execute.  Score matmuls of adjacent heads alternate between
the two 64-row halves of the PE array (tile_position row packing) so they
run concurrently on hardware.
"""

import sys

if "/opt/trn_rl_repo" not in sys.path:
    sys.path.insert(0, "/opt/trn_rl_repo")

import numpy as np

import concourse.bacc as bacc
import concourse.mybir as mybir
import concourse.tile as tile
from concourse.bass_utils import run_bass_kernel_spmd

N_CORES = 8
B = 4
TQ = 1024
TK = 2048
D = 1024
H = 16
HD = 64
H_LOC = 8          # heads per core
DH = H_LOC * HD    # 512: per-core head dims
F32 = mybir.dt.float32
F32R = mybir.dt.float32r
EXP = mybir.ActivationFunctionType.Exp

_PROGRAM_CACHE = {}


def _build_program(trivial_mask, hpg=4, ps_bufs=2, av_bufs=4, interleave=True):
    nc = bacc.Bacc()

    xT = nc.dram_tensor("xT", [D, TQ], F32R, kind="ExternalInput").ap()
    memT = nc.dram_tensor("memT", [D, TK], F32R, kind="ExternalInput").ap()
    wq_d = nc.dram_tensor("wq", [D, DH], F32R, kind="ExternalInput").ap()
    wk_d = nc.dram_tensor("wk", [D, DH], F32R, kind="ExternalInput").ap()
    wv_d = nc.dram_tensor("wv", [D, DH], F32R, kind="ExternalInput").ap()
    wo_d = nc.dram_tensor("wo", [DH, D], F32R, kind="ExternalInput").ap()
    bq_d = nc.dram_tensor("bq", [DH], F32, kind="ExternalInput").ap()
    bk_d = nc.dram_tensor("bk", [DH], F32, kind="ExternalInput").ap()
    bv_d = nc.dram_tensor("bv", [DH], F32, kind="ExternalInput").ap()
    maskb_d = nc.dram_tensor("maskb", [TK], F32, kind="ExternalInput").ap()
    ones_d = nc.dram_tensor("ones_in", [64], F32R, kind="ExternalInput").ap()
    y_d = nc.dram_tensor("y", [TQ, D], F32, kind="ExternalOutput").ap()

    with tile.TileContext(nc, pool_alloc_mode="queue") as tc, \
            nc.allow_low_precision(reason="float32r operands; fp32 PSUM accum"):
        # ---- constants / biases ------------------------------------------
        singles = tc.alloc_tile_pool(name="singles", bufs=1)
        bq_sb = singles.tile([128, 4], F32, tag="bq")
        nc.sync.dma_start(out=bq_sb, in_=bq_d.rearrange("(c p) -> p c", p=128))
        bk_sb = singles.tile([128, 4], F32, tag="bk")
        nc.sync.dma_start(out=bk_sb, in_=bk_d.rearrange("(c p) -> p c", p=128))
        maskb_sb = singles.tile([128, 16], F32, tag="maskb")
        nc.sync.dma_start(out=maskb_sb, in_=maskb_d.rearrange("(c p) -> p c", p=128))
        bv_bc = singles.tile([128, DH], F32, tag="bv")
        nc.gpsimd.dma_start(out=bv_bc, in_=bv_d.partition_broadcast(128))
        ones64 = singles.tile([1, 64], F32R, tag="ones64")
        nc.sync.dma_start(out=ones64, in_=ones_d.rearrange("(p n) -> p n", p=1))
        onescol = singles.tile([128, H_LOC, 1], F32R, tag="onescol")
        nc.sync.dma_start(
            out=onescol,
            in_=ones_d.partition_broadcast(128)[:, 0:H_LOC].rearrange(
                "p (n u) -> p n u", u=1
            ),
        )

        # ---- persistent activations --------------------------------------
        p_qt = tc.alloc_tile_pool(name="qt", bufs=1)
        Qt = [p_qt.tile([128, TQ], F32R, tag=f"qt{i}", name=f"qt{i}")
              for i in range(4)]
        p_kt = tc.alloc_tile_pool(name="kt", bufs=1)
        Kt = [p_kt.tile([128, TK], F32R, tag=f"kt{i}", name=f"kt{i}")
              for i in range(4)]
        p_v = tc.alloc_tile_pool(name="v", bufs=1)
        V = [p_v.tile([128, H_LOC, HD + 1], F32R, tag=f"v{i}", name=f"v{i}")
             for i in range(16)]
        p_ctx = tc.alloc_tile_pool(name="ctx", bufs=1)
        ctxT = [p_ctx.tile([128, TQ], F32R, tag=f"ctx{i}", name=f"ctx{i}")
                for i in range(4)]
        p_in = tc.alloc_tile_pool(name="inp", bufs=2)
        p_st = tc.alloc_tile_pool(name="st", bufs=3)
        p_cun = tc.alloc_tile_pool(name="cun", bufs=4)
        p_small = tc.alloc_tile_pool(name="small", bufs=2)

        # Two PSUM pools for the whole program: "ps" (2 x 2-bank slots) for
        # projections / score pairs / broadcasts, "av" (4 x 1-bank slots)
        # for the per-head attention accumulators.
        p_ps = tc.alloc_tile_pool(name="ps", bufs=ps_bufs, space="PSUM")
        p_av = tc.alloc_tile_pool(name="av", bufs=av_bufs, space="PSUM")

        def ps_tile(shape, name):
            return p_ps.tile(shape, F32, tag="ps", name=name,
                             padded_shape=[128, 1024])

        # ---- Q projection: Qt[dq, t] = sum_d W_q[d, dq] x^T[d, t] + b_q --
        p_wq = tc.alloc_tile_pool(name="wq", bufs=1)
        wq_r = wq_d.rearrange("(c p) n -> c p n", p=128)
        wq_sb = []
        for c in range(8):
            t = p_wq.tile([128, DH], F32R, tag=f"wq{c}", name=f"wq{c}")
            nc.sync.dma_start(out=t, in_=wq_r[c])
            wq_sb.append(t)

        for tt in range(2):
            xin = []
            for c in range(8):
                t = p_in.tile([128, 512], F32R, tag=f"in{c}", name=f"inx{c}")
                nc.sync.dma_start(
                    out=t, in_=xT[c * 128:(c + 1) * 128, tt * 512:(tt + 1) * 512]
                )
                xin.append(t)
            for dqc in range(4):
                ps = ps_tile([128, 512], f"ps_q{tt}{dqc}")
                for c in range(8):
                    nc.tensor.matmul(
                        ps,
                        lhsT=wq_sb[c][:, dqc * 128:(dqc + 1) * 128],
                        rhs=xin[c],
                        start=(c == 0),
                        stop=(c == 7),
                    )
                nc.vector.tensor_scalar_add(
                    out=Qt[dqc][:, tt * 512:(tt + 1) * 512],
                    in0=ps,
                    scalar1=bq_sb[:, dqc:dqc + 1],
                )
        p_wq.release()

        # ---- K/V projection (one k-tile of 512 keys) ----------------------
        p_wkv = tc.alloc_tile_pool(name="wkv", bufs=1)
        wk_r = wk_d.rearrange("(c p) n -> c p n", p=128)
        wv_r = wv_d.rearrange("(c p) n -> c p n", p=128)
        wk_sb, wv_sb = [], []
        for c in range(8):
            t = p_wkv.tile([128, DH], F32R, tag=f"wk{c}", name=f"wk{c}")
            nc.sync.dma_start(out=t, in_=wk_r[c])
            wk_sb.append(t)
            t = p_wkv.tile([128, DH], F32R, tag=f"wv{c}", name=f"wv{c}")
            nc.sync.dma_start(out=t, in_=wv_r[c])
            wv_sb.append(t)

        def kv_tile(kt):
            min_ = []
            for c in range(8):
                t = p_in.tile([128, 512], F32R, tag=f"in{c}", name=f"inm{c}")
                nc.sync.dma_start(
                    out=t, in_=memT[c * 128:(c + 1) * 128, kt * 512:(kt + 1) * 512]
                )
                min_.append(t)
            for dkc in range(4):
                ps = ps_tile([128, 512], f"ps_k{kt}{dkc}")
                for c in range(8):
                    nc.tensor.matmul(
                        ps,
                        lhsT=wk_sb[c][:, dkc * 128:(dkc + 1) * 128],
                        rhs=min_[c],
                        start=(c == 0),
                        stop=(c == 7),
                    )
                nc.vector.tensor_scalar_add(
                    out=Kt[dkc][:, kt * 512:(kt + 1) * 512],
                    in0=ps,
                    scalar1=bk_sb[:, dkc:dkc + 1],
                )
            for j in range(4):
                kk = kt * 4 + j
                ps = ps_tile([128, 512], f"ps_v{kk}")
                for c in range(8):
                    nc.tensor.matmul(
                        ps,
                        lhsT=min_[c][:, j * 128:(j + 1) * 128],
                        rhs=wv_sb[c],
                        start=(c == 0),
                        stop=(c == 7),
                    )
                vt = V[kk]
                nc.vector.tensor_add(
                    out=vt[:, :, 0:HD],
                    in0=ps.rearrange("p (h e) -> p h e", h=H_LOC),
                    in1=bv_bc.rearrange("p (h e) -> p h e", h=H_LOC),
                )
                nc.vector.tensor_copy(out=vt[:, :, HD:HD + 1], in_=onescol)

        # ---- attention helpers --------------------------------------------
        def att_alloc_avs(qt_i, hg):
            return {
                h: p_av.tile([65, 512], F32, tag="av", name=f"av{h}_{qt_i}")
                for h in [hg * hpg + i for i in range(hpg)]
            }

        def att_pairs(qt_i, hg, avs, pps):
            qsl = slice(qt_i * 512, (qt_i + 1) * 512)
            for pp in pps:
                kks = (2 * pp, 2 * pp + 1)
                for h in [hg * hpg + i for i in range(hpg)]:
                    ht, hb = h // 2, (h % 2) * 64
                    ps = ps_tile([128, 1024], f"sc{h}_{pp}_{qt_i}")
                    for half, kk in enumerate(kks):
                        nc.tensor.matmul(
                            ps[:, half * 512:(half + 1) * 512],
                            lhsT=Kt[ht][hb:hb + 64, kk * 128:(kk + 1) * 128],
                            rhs=Qt[ht][hb:hb + 64, qsl],
                            start=True,
                            stop=True,
                            tile_position=(hb, 0),
                        )
                    st = p_st.tile([128, 1024], F32R, tag="st",
                                   name=f"st{h}_{pp}")
                    if trivial_mask:
                        nc.scalar.activation(
                            out=st, in_=ps, func=EXP, bias=0.0, scale=0.125
                        )
                    else:
                        for half, kk in enumerate(kks):
                            nc.scalar.activation(
                                out=st[:, half * 512:(half + 1) * 512],
                                in_=ps[:, half * 512:(half + 1) * 512],
                                func=EXP,
                                bias=maskb_sb[:, kk:kk + 1],
                                scale=0.125,
                            )
                    for half, kk in enumerate(kks):
                        nc.tensor.matmul(
                            avs[h],
                            lhsT=V[kk][:, h, :],
                            rhs=st[:, half * 512:(half + 1) * 512],
                            start=(kk == 0),
                            stop=(kk == 15),
                        )

        def att_norm(qt_i, avs):
            qsl = slice(qt_i * 512, (qt_i + 1) * 512)
            cuns = {}
            for h, av in avs.items():
                cun = p_cun.tile([65, 512], F32, tag="cun",
                                 name=f"cun{h}_{qt_i}")
                nc.vector.tensor_copy(out=cun, in_=av)
                cuns[h] = cun
            for h, cun in cuns.items():
                ht, hb = h // 2, (h % 2) * 64
                recip = p_small.tile([1, 512], F32R, tag="recip",
                                     name=f"recip{h}")
                nc.vector.reciprocal(out=recip, in_=cun[64:65, :])
                rb_ps = p_av.tile([64, 512], F32, tag="av", name=f"rb_ps{h}")
                nc.tensor.matmul(rb_ps, lhsT=ones64, rhs=recip,
                                 start=True, stop=True)
                rb = p_small.tile([64, 512], F32, tag="rb", name=f"rb{h}")
                nc.vector.tensor_copy(out=rb, in_=rb_ps)
                nc.vector.tensor_mul(
                    out=ctxT[ht][hb:hb + 64, qsl], in0=cun[0:64, :], in1=rb
                )

        def out_proj(p_y, wo_sb, qcs):
            for qc in qcs:
                ysb = p_y.tile([128, D], F32, tag="y", name=f"y{qc}")
                for ot in range(2):
                    ps = ps_tile([128, 512], f"ps_o{qc}{ot}")
                    for c in range(4):
                        nc.tensor.matmul(
                            ps,
                            lhsT=ctxT[c][:, qc * 128:(qc + 1) * 128],
                            rhs=wo_sb[c][:, ot * 512:(ot + 1) * 512],
                            start=(c == 0),
                            stop=(c == 3),
                        )
                    nc.vector.tensor_copy(
                        out=ysb[:, ot * 512:(ot + 1) * 512], in_=ps
                    )
                nc.sync.dma_start(out=y_d[qc * 128:(qc + 1) * 128, :], in_=ysb)

        # ---- schedule -----------------------------------------------------
        n_groups = H_LOC // hpg
        kv_tile(0)
        kv_tile(1)

        g0 = att_alloc_avs(0, 0)
        if interleave:
            att_pairs(0, 0, g0, range(0, 4))  # kk 0..7 need only k-tiles 0,1

        kv_tile(2)
        kv_tile(3)
        p_wkv.release()

        p_wo = tc.alloc_tile_pool(name="wo", bufs=1)
        wo_r = wo_d.rearrange("(c p) n -> c p n", p=128)
        wo_sb = []
        for c in range(4):
            t = p_wo.tile([128, D], F32R, tag=f"wo{c}", name=f"wo{c}")
            nc.sync.dma_start(out=t, in_=wo_r[c])
            wo_sb.append(t)
        p_y = tc.alloc_tile_pool(name="y", bufs=2)

        att_pairs(0, 0, g0, range(4, 8) if interleave else range(8))
        att_norm(0, g0)
        for hg in range(1, n_groups):
            g = att_alloc_avs(0, hg)
            att_pairs(0, hg, g, range(8))
            att_norm(0, g)

        out_proj(p_y, wo_sb, range(0, 4))    # q rows 0..511 (qt 0)

        for hg in range(n_groups):
            g = att_alloc_avs(1, hg)
            att_pairs(1, hg, g, range(8))
            att_norm(1, g)

        out_proj(p_y, wo_sb, range(4, 8))    # q rows 512..1023 (qt 1)

        for pool in (p_y, p_wo, p_av, p_ps, p_small, p_cun, p_st, p_in,
                     p_ctx, p_v, p_kt, p_qt, singles):
            pool.release()

    nc.compile()
    return nc


BUILD_OPTS = dict(hpg=2, ps_bufs=3, av_bufs=2, interleave=True)


def get_program(trivial_mask=True):
    key = ("nc", bool(trivial_mask), tuple(sorted(BUILD_OPTS.items())))
    if key not in _PROGRAM_CACHE:
        _PROGRAM_CACHE[key] = _build_program(trivial_mask, **BUILD_OPTS)
    return _PROGRAM_CACHE[key]


def make_in_maps(x, memory, memory_padding_mask, W_q, b_q, W_kv, b_kv, W_o):
    x = np.asarray(x, dtype=np.float32)
    memory = np.asarray(memory, dtype=np.float32)
    mask = np.asarray(memory_padding_mask)
    W_q = np.asarray(W_q, dtype=np.float32)
    b_q = np.asarray(b_q, dtype=np.float32)
    W_kv = np.asarray(W_kv, dtype=np.float32)
    b_kv = np.asarray(b_kv, dtype=np.float32)
    W_o = np.asarray(W_o, dtype=np.float32)

    in_maps = []
    for c in range(N_CORES):
        b, s = c // 2, c % 2
        sl = slice(s * DH, (s + 1) * DH)
        vsl = slice(D + s * DH, D + (s + 1) * DH)
        in_maps.append({
            "xT": np.ascontiguousarray(x[b].T),
            "memT": np.ascontiguousarray(memory[b].T),
            "wq": np.ascontiguousarray(W_q[:, sl]),
            "wk": np.ascontiguousarray(W_kv[:, sl]),
            "wv": np.ascontiguousarray(W_kv[:, vsl]),
            "wo": np.ascontiguousarray(W_o[sl, :]),
            "bq": np.ascontiguousarray(b_q[sl]),
            "bk": np.ascontiguousarray(b_kv[sl]),
            "bv": np.ascontiguousarray(b_kv[vsl]),
            "maskb": np.where(mask[b], 0.0, -30000.0).astype(np.float32),
            "ones_in": np.ones(64, dtype=np.float32),
        })
    return in_maps


def kernel(x, memory, memory_padding_mask, W_q, b_q, W_kv, b_kv, W_o, b_o):
    trivial_mask = bool(np.asarray(memory_padding_mask).all())
    nc = get_program(trivial_mask)
    in_maps = make_in_maps(
        x, memory, memory_padding_mask, W_q, b_q, W_kv, b_kv, W_o
    )
    res = run_bass_kernel_spmd(nc, in_maps, list(range(N_CORES)))
    ys = [res.results[c]["y"] for c in range(N_CORES)]
    b_o = np.asarray(b_o, dtype=np.float32)
    out = np.stack([ys[2 * b] + ys[2 * b + 1] for b in range(B)])
    out += b_o[None, None, :]
    return out.astype(np.float32)
